# revision 6
# baseline (speedup 1.0000x reference)
"""CrossNet layer kernel for Trainium2 (8 NeuronCores, data parallel).

Computes: out = X * (X @ alphas)[:, None] + bias + X
        = X * (1 + X @ alphas)[:, None] + bias

X: [16384, 4096] f32, alphas: [4096] f32, bias: [4096] f32.

Sharding: X split along batch into 8 row-shards of [2048, 4096]; alphas/bias
replicated (pre-broadcast on host to [128, D] so each core loads them once).

Per [128, 4096] tile on each core:
  1. DVE tensor_tensor_reduce: scratch = X*A ; s = 1 + sum_free(X*A)   (one pass)
  2. bias == 0 (fast path): ACT activation(Copy, scale=s): out = X*s   (ACT pass)
     bias != 0: DVE scalar_tensor_tensor: out = (X * s) + B_rep        (one pass)
  3. DMA out.
DMA is the bottleneck (~64 MiB HBM traffic per core @ ~358 GB/s ~= 187 us).
"""

import os
import sys

for _p in ("/opt/trn_rl_repo",):
    if _p not in sys.path and os.path.isdir(_p):
        sys.path.insert(0, _p)

import numpy as np

import concourse.bacc as bacc
import concourse.bass as bass
import concourse.mybir as mybir
from concourse.bass_utils import run_bass_kernel_spmd
from concourse.tile import TileContext

N_CORES = 8
B_FULL = 16384
D = 4096
R = B_FULL // N_CORES  # rows per core
P = 128  # partitions

_CACHE = {}


def _build(has_bias: bool) -> bass.Bass:
    f32 = mybir.dt.float32
    nc = bacc.Bacc("TRN2", target_bir_lowering=False)
    x = nc.dram_tensor("x", (R, D), f32, kind="ExternalInput")
    arep = nc.dram_tensor("arep", (P, D), f32, kind="ExternalInput")
    if has_bias:
        brep = nc.dram_tensor("brep", (P, D), f32, kind="ExternalInput")
    out = nc.dram_tensor("out", (R, D), f32, kind="ExternalOutput")

    n_tiles = R // P
    mult = mybir.AluOpType.mult
    add = mybir.AluOpType.add
    bypass = mybir.AluOpType.bypass

    with TileContext(nc) as tc:
        with tc.tile_pool(name="const", bufs=1) as cpool:
            a_t = cpool.tile([P, D], f32)
            nc.sync.dma_start(out=a_t, in_=arep[:, :])
            if has_bias:
                b_t = cpool.tile([P, D], f32)
                nc.sync.dma_start(out=b_t, in_=brep[:, :])
            with tc.tile_pool(name="work", bufs=3) as pool:
                for i in range(n_tiles):
                    rows = slice(i * P, (i + 1) * P)
                    x_t = pool.tile([P, D], f32, tag="x")
                    nc.sync.dma_start(out=x_t, in_=x[rows, :])
                    scr = pool.tile([P, D], f32, tag="scr", bufs=2)
                    s_t = pool.tile([P, 1], f32, tag="s", bufs=2)
                    s1_t = pool.tile([P, 1], f32, tag="s1", bufs=2)
                    # scr = (x bypass _) * a = x*a ; s = sum_free(x*a)
                    # (fused multiply-reduce in one DVE pass; the plain
                    # tensor_tensor_reduce ISA op crashes this runtime)
                    nc.vector.scalar_tensor_tensor(
                        out=scr,
                        in0=x_t,
                        scalar=0.0,
                        in1=a_t,
                        op0=bypass,
                        op1=mult,
                        accum_out=s_t,
                    )
                    # s1 = 1 + x.a   (folds the "+ X" residual term)
                    nc.vector.tensor_scalar_add(out=s1_t, in0=s_t, scalar1=1.0)
                    o_t = pool.tile([P, D], f32, tag="o")
                    if has_bias:
                        nc.vector.scalar_tensor_tensor(
                            out=o_t,
                            in0=x_t,
                            scalar=s1_t,
                            in1=b_t,
                            op0=mult,
                            op1=add,
                        )
                    else:
                        nc.scalar.mul(o_t, x_t, s1_t)
                    nc.sync.dma_start(out=out[rows, :], in_=o_t)
    nc.compile()
    return nc


def _run(X, alphas, bias, trace=False, trace_kwargs=None):
    X = np.ascontiguousarray(np.asarray(X, dtype=np.float32))
    alphas = np.asarray(alphas, dtype=np.float32)
    bias = np.asarray(bias, dtype=np.float32)
    assert X.shape == (B_FULL, D), X.shape

    has_bias = bool(np.any(bias))
    if has_bias not in _CACHE:
        _CACHE[has_bias] = _build(has_bias)
    nc = _CACHE[has_bias]

    arep = np.ascontiguousarray(np.broadcast_to(alphas, (P, D)))
    in_maps = []
    for c in range(N_CORES):
        m = {"x": np.ascontiguousarray(X[c * R : (c + 1) * R]), "arep": arep}
        if has_bias:
            m["brep"] = np.ascontiguousarray(np.broadcast_to(bias, (P, D)))
        in_maps.append(m)

    res = run_bass_kernel_spmd(
        nc,
        in_maps,
        core_ids=list(range(N_CORES)),
        trace=trace,
        **(trace_kwargs or {}),
    )
    full = np.concatenate([r["out"] for r in res.results], axis=0)
    return full, res


def kernel(X, alphas, bias):
    out, _ = _run(X, alphas, bias, trace=False)
    return out


# revision 7
# speedup vs baseline: 1.2007x; 1.2007x over previous
"""CrossNet layer kernel for Trainium2 (8 NeuronCores, data parallel).

Computes: out = X * (X @ alphas)[:, None] + bias + X
        = X * (1 + X @ alphas)[:, None] + bias

X: [16384, 4096] f32, alphas: [4096] f32, bias: [4096] f32.

Sharding: X split along batch into 8 row-shards of [2048, 4096]; alphas/bias
replicated (tiny, loaded once per core and broadcast across partitions
on-chip so no replicated DRAM traffic).

Per [128, 4096] tile on each core:
  1. DVE scalar_tensor_tensor: scr = (X bypass _) * A, accum s = sum(X*A)
     (fused multiply+row-reduce in one DVE pass)
  2. DVE tensor_scalar_add:    s1 = 1 + s        ([128,1], folds the +X term)
  3. bias == 0 (fast path): ACT activation(Copy, scale=s1): out = X*s1
     bias != 0: DVE scalar_tensor_tensor: out = (X * s1) + B_rep
  4. DMA out — deferred by 2 iterations so the single HWDGE queue still has
     store work while the final tile's compute runs (kills the tail bubble).
DMA is the bottleneck: 64 MiB of HBM traffic per core @ ~358 GB/s ~= 190 us.
"""

import os
import sys

for _p in ("/opt/trn_rl_repo",):
    if _p not in sys.path and os.path.isdir(_p):
        sys.path.insert(0, _p)

import numpy as np

import concourse.bacc as bacc
import concourse.bass as bass
import concourse.mybir as mybir
from concourse.bass_utils import run_bass_kernel_spmd
from concourse.tile import TileContext

N_CORES = 8
B_FULL = 16384
D = 4096
R = B_FULL // N_CORES  # rows per core
P = 128  # partitions

# Stores lag their producing iteration by this many iterations.
STORE_LAG = 2

_CACHE = {}


def _build(has_bias: bool) -> bass.Bass:
    f32 = mybir.dt.float32
    nc = bacc.Bacc("TRN2", target_bir_lowering=False)
    x = nc.dram_tensor("x", (R, D), f32, kind="ExternalInput")
    a0 = nc.dram_tensor("a0", (1, D), f32, kind="ExternalInput")
    if has_bias:
        b0 = nc.dram_tensor("b0", (1, D), f32, kind="ExternalInput")
    out = nc.dram_tensor("out", (R, D), f32, kind="ExternalOutput")

    n_tiles = R // P
    mult = mybir.AluOpType.mult
    add = mybir.AluOpType.add
    bypass = mybir.AluOpType.bypass

    with TileContext(nc) as tc:
        with tc.tile_pool(name="const", bufs=1) as cpool:
            a0_t = cpool.tile([1, D], f32)
            nc.sync.dma_start(out=a0_t, in_=a0[:, :])
            a_t = cpool.tile([P, D], f32)
            nc.gpsimd.partition_broadcast(a_t, a0_t)
            if has_bias:
                b0_t = cpool.tile([1, D], f32)
                nc.sync.dma_start(out=b0_t, in_=b0[:, :])
                b_t = cpool.tile([P, D], f32)
                nc.gpsimd.partition_broadcast(b_t, b0_t)
            with tc.tile_pool(name="work", bufs=3) as pool:
                pending = []

                def flush_one():
                    j, o = pending.pop(0)
                    nc.sync.dma_start(
                        out=out[j * P : (j + 1) * P, :], in_=o
                    )

                for i in range(n_tiles):
                    rows = slice(i * P, (i + 1) * P)
                    x_t = pool.tile([P, D], f32, tag="x", bufs=4)
                    nc.sync.dma_start(out=x_t, in_=x[rows, :])
                    scr = pool.tile([P, D], f32, tag="scr", bufs=1)
                    s_t = pool.tile([P, 1], f32, tag="s", bufs=2)
                    s1_t = pool.tile([P, 1], f32, tag="s1", bufs=2)
                    # scr = (x bypass _) * a = x*a ; s = sum_free(x*a)
                    nc.vector.scalar_tensor_tensor(
                        out=scr,
                        in0=x_t,
                        scalar=0.0,
                        in1=a_t,
                        op0=bypass,
                        op1=mult,
                        accum_out=s_t,
                    )
                    # s1 = 1 + x.a   (folds the "+ X" residual term)
                    nc.vector.tensor_scalar_add(out=s1_t, in0=s_t, scalar1=1.0)
                    o_t = pool.tile([P, D], f32, tag="o", bufs=STORE_LAG + 3)
                    if has_bias:
                        nc.vector.scalar_tensor_tensor(
                            out=o_t,
                            in0=x_t,
                            scalar=s1_t,
                            in1=b_t,
                            op0=mult,
                            op1=add,
                        )
                    else:
                        nc.scalar.mul(o_t, x_t, s1_t)
                    pending.append((i, o_t))
                    if len(pending) > STORE_LAG:
                        flush_one()
                while pending:
                    flush_one()
    nc.compile()
    return nc


def _run(X, alphas, bias, trace=False, trace_kwargs=None):
    X = np.ascontiguousarray(np.asarray(X, dtype=np.float32))
    alphas = np.asarray(alphas, dtype=np.float32)
    bias = np.asarray(bias, dtype=np.float32)
    assert X.shape == (B_FULL, D), X.shape

    has_bias = bool(np.any(bias))
    if has_bias not in _CACHE:
        _CACHE[has_bias] = _build(has_bias)
    nc = _CACHE[has_bias]

    a0 = np.ascontiguousarray(alphas.reshape(1, D))
    in_maps = []
    for c in range(N_CORES):
        m = {"x": np.ascontiguousarray(X[c * R : (c + 1) * R]), "a0": a0}
        if has_bias:
            m["b0"] = np.ascontiguousarray(bias.reshape(1, D))
        in_maps.append(m)

    res = run_bass_kernel_spmd(
        nc,
        in_maps,
        core_ids=list(range(N_CORES)),
        trace=trace,
        **(trace_kwargs or {}),
    )
    full = np.concatenate([r["out"] for r in res.results], axis=0)
    return full, res


def kernel(X, alphas, bias):
    out, _ = _run(X, alphas, bias, trace=False)
    return out
